# revision 3
# baseline (speedup 1.0000x reference)
"""Bidirectional-LSTM Trainium2 kernel (nn_BLSTM).

Problem: B=64,T=512,D=H=512. Two independent LSTMs (forward input x_f,
backward input x_b), outputs summed, then two H x H linears collapsed
into one (W21 = W2 @ W1, b21 = W2 @ b1 + b2).

Sharding (8 cores, SPMD): core r: direction r % 2, batch shard r // 2;
each core runs one direction for BL=16 batches; host sums partial outs.

Step design:
  - xg (input projection + bias, precomputed per 32-step chunk) is
    accumulated into the gate PSUM tile by an identity-weight matmul
    issued before the recurrence matmuls (start=True clears the bank),
    so there is no DVE "psum + xg" add and the activations read PSUM.
  - gate column layout [i | f | g | o]; matmul order i,f then g then o,
    so sigmoid(i,f) starts while g/o matmuls still stream, and the o
    sigmoid sits off the critical path.
  - true Tanh table for g and c (Sigmoid/Tanh/Identity share one
    activation table set), cutting the DVE chain to 4 ops:
    c1 = sig_f*c, t1 = sig_i*tanh_g, c = c1+t1, h = sig_o*tanh(c).
  - proj/linear PSUM evacuations are emitted after the chain so they
    never sit ahead of the step's sigmoid in the ACT FIFO.
"""

import functools
import numpy as np
import ml_dtypes

import concourse.bass as bass
import concourse.tile as tile
from concourse import bacc, mybir
from concourse.bass_utils import run_bass_kernel_spmd

# ---------------- problem constants ----------------
B, T, D, H = 64, 512, 512, 512
G = 4 * H                 # 2048 gate dim
N_CORES = 8
BL = B // (N_CORES // 2)  # 16 local batch per core
TC = 32                   # timesteps per chunk
NCH = T // TC             # chunks

F32 = mybir.dt.float32
BF16 = mybir.dt.bfloat16
AFT = mybir.ActivationFunctionType


def _build_program(chunks=None):
    if chunks is None:
        chunks = NCH
    wdt = BF16
    nc = bacc.Bacc("TRN2", target_bir_lowering=False, debug=False,
                   num_devices=N_CORES)

    xT_d = nc.dram_tensor("xT", [4, 128, T, BL], wdt, kind="ExternalInput").ap()
    wih_d = nc.dram_tensor("wih", [4, 128, G], wdt, kind="ExternalInput").ap()
    whh_d = nc.dram_tensor("whh", [4, 128, G], wdt, kind="ExternalInput").ap()
    w21_d = nc.dram_tensor("w21", [4, 128, H], wdt, kind="ExternalInput").ap()
    biasg_d = nc.dram_tensor("biasg", [128, 16], F32, kind="ExternalInput").ap()
    b21_d = nc.dram_tensor("b21", [128, 4], F32, kind="ExternalInput").ap()
    h0_d = nc.dram_tensor("h0p", [128, 64], wdt, kind="ExternalInput").ap()
    c0_d = nc.dram_tensor("c0p", [128, 64], F32, kind="ExternalInput").ap()
    id_d = nc.dram_tensor("ident", [128, 128], wdt, kind="ExternalInput").ap()
    pred_d = nc.dram_tensor("predT", [H, T * BL], F32, kind="ExternalOutput").ap()

    with tile.TileContext(nc) as tc:
        with (
            tc.tile_pool(name="const", bufs=1) as cpool,
            tc.tile_pool(name="xch", bufs=2) as xch_pool,
            tc.tile_pool(name="xg", bufs=2) as xg_pool,
            tc.tile_pool(name="ring", bufs=2) as ring_pool,
            tc.tile_pool(name="aif", bufs=3) as aif_pool,
            tc.tile_pool(name="small", bufs=4) as small_pool,
            tc.tile_pool(name="cstate", bufs=2) as c_pool,
            tc.tile_pool(name="evac", bufs=2) as evac_pool,
            tc.tile_pool(name="gps", bufs=2, space="PSUM") as gps_pool,
            tc.tile_pool(name="pps", bufs=2, space="PSUM") as pps_pool,
            tc.tile_pool(name="lps", bufs=2, space="PSUM") as lps_pool,
        ):
            # ---- preload constants ----
            whh_sb = cpool.tile([128, 4 * G], wdt, tag="whh")
            wih_sb = cpool.tile([128, 4 * G], wdt, tag="wih")
            w21_sb = cpool.tile([128, 4 * H], wdt, tag="w21")
            ident_sb = cpool.tile([128, 128], wdt, tag="ident")
            biasg_sb = cpool.tile([128, 16], F32, tag="biasg")
            b21_sb = cpool.tile([128, 4], F32, tag="b21")
            h0_sb = cpool.tile([128, 64], wdt, tag="h0")
            c0_sb = cpool.tile([128, 64], F32, tag="c0")
            for kc in range(4):
                nc.gpsimd.dma_start(whh_sb[:, kc * G:(kc + 1) * G], whh_d[kc])
                nc.gpsimd.dma_start(wih_sb[:, kc * G:(kc + 1) * G], wih_d[kc])
                nc.gpsimd.dma_start(w21_sb[:, kc * H:(kc + 1) * H], w21_d[kc])
            nc.gpsimd.dma_start(ident_sb[:], id_d[:])
            nc.gpsimd.dma_start(biasg_sb[:], biasg_d[:])
            nc.gpsimd.dma_start(b21_sb[:], b21_d[:])
            nc.gpsimd.dma_start(h0_sb[:], h0_d[:])
            nc.gpsimd.dma_start(c0_sb[:], c0_d[:])

            # ---- projection helpers ----
            def proj_dma(ch):
                xch = xch_pool.tile([128, 4 * TC * BL], wdt, tag="xch")
                for dc in range(4):
                    nc.gpsimd.dma_start(
                        xch[:, dc * TC * BL:(dc + 1) * TC * BL],
                        xT_d[dc, :, ch * TC:(ch + 1) * TC, :])
                return xch

            def proj_mms(xch, jc):
                pp = pps_pool.tile([128, TC * BL], F32, tag="pp")
                for dc in range(4):
                    nc.tensor.matmul(
                        pp[:],
                        wih_sb[:, dc * G + jc * 128: dc * G + (jc + 1) * 128],
                        xch[:, dc * TC * BL:(dc + 1) * TC * BL],
                        start=(dc == 0), stop=(dc == 3))
                return pp

            def proj_evac(pp, xg, jc):
                off = (jc // 4) * 64 + (jc % 4) * 16
                dst = xg[:].rearrange("p (t c) -> p t c", c=256)[:, :, off:off + 16]
                nc.scalar.activation(dst, pp[:], AFT.Identity,
                                     bias=biasg_sb[:, jc:jc + 1])

            def linear_mms(ring_src, jc):
                lp = lps_pool.tile([128, TC * BL], F32, tag="lp")
                r3 = ring_src[:].rearrange("p (t c) -> p t c", c=64)
                for kc in range(4):
                    nc.tensor.matmul(
                        lp[:],
                        w21_sb[:, kc * H + jc * 128: kc * H + (jc + 1) * 128],
                        r3[:, :, kc * 16:(kc + 1) * 16],
                        start=(kc == 0), stop=(kc == 3))
                return lp

            def linear_evac(lp, ch_src, jc):
                ev = evac_pool.tile([128, TC * BL], F32, tag="ev")
                nc.scalar.activation(ev[:], lp[:], AFT.Identity,
                                     bias=b21_sb[:, jc:jc + 1])
                nc.gpsimd.dma_start(
                    pred_d[jc * 128:(jc + 1) * 128,
                           ch_src * TC * BL:(ch_src + 1) * TC * BL], ev[:])

            # ---- prologue: project chunk 0 ----
            xch = proj_dma(0)
            xg_cur = xg_pool.tile([128, TC * 256], wdt, tag="xg")
            for jc in range(16):
                proj_evac(proj_mms(xch, jc), xg_cur, jc)

            c_prev = c0_sb
            prev_ring = None
            xg_next = None
            pend_evacs = []   # deferred ACT evacuations (emit after chain)
            for ch in range(chunks):
                ring = ring_pool.tile([128, TC * 64], wdt, tag="ring")
                for tl in range(TC):
                    if tl > 0:
                        hsrc, hoff = ring, (tl - 1) * 64
                    elif ch > 0:
                        hsrc, hoff = prev_ring, (TC - 1) * 64
                    else:
                        hsrc, hoff = h0_sb, 0
                    # ---- gate psum: xg preload + recurrence matmuls ----
                    # col layout: [i(0:64) | f(64:128) | g(128:192) | o(192:256)]
                    gps = gps_pool.tile([128, 256], F32, tag="gps")
                    nc.tensor.matmul(gps[:], ident_sb[:],
                                     xg_cur[:, tl * 256:(tl + 1) * 256],
                                     start=True, stop=False,
                                     skip_group_check=True)
                    # emission order i, f, g, o: sigmoid(i,f) starts after
                    # 32 matmuls, and the o-block matmuls + sigmoid(o) sit
                    # entirely off the critical path (h needs them last).
                    for g_idx in (0, 1, 2, 3):   # i, f, g, o
                        for hc in range(4):
                            jc = g_idx * 4 + hc
                            off = g_idx * 64 + hc * 16
                            for kc in range(4):
                                nc.tensor.matmul(
                                    gps[:, off:off + 16],
                                    whh_sb[:, kc * G + jc * 128: kc * G + (jc + 1) * 128],
                                    hsrc[:, hoff + kc * 16: hoff + (kc + 1) * 16],
                                    start=False,
                                    stop=(hc == 3 and kc == 3),
                                    skip_group_check=True)
                    # ---- filler work (PE; runs during this step's chain) ----
                    if ch + 1 < chunks:
                        if tl == 0:
                            xch = proj_dma(ch + 1)
                            xg_next = xg_pool.tile([128, TC * 256], wdt, tag="xg")
                        jc_f = tl // 2
                        if tl % 2 == 0:
                            pp_cur = pps_pool.tile([128, TC * BL], F32, tag="pp")
                            for dc in (0, 1):
                                nc.tensor.matmul(
                                    pp_cur[:],
                                    wih_sb[:, dc * G + jc_f * 128: dc * G + (jc_f + 1) * 128],
                                    xch[:, dc * TC * BL:(dc + 1) * TC * BL],
                                    start=(dc == 0), stop=False)
                        else:
                            for dc in (2, 3):
                                nc.tensor.matmul(
                                    pp_cur[:],
                                    wih_sb[:, dc * G + jc_f * 128: dc * G + (jc_f + 1) * 128],
                                    xch[:, dc * TC * BL:(dc + 1) * TC * BL],
                                    start=False, stop=(dc == 3))
                            pend_evacs.append(
                                (proj_evac, (pp_cur, xg_next, jc_f)))
                    if ch >= 1 and tl in (3, 11, 19, 27):
                        jc_l = (tl - 3) // 8
                        lp = linear_mms(prev_ring, jc_l)
                        pend_evacs.append((linear_evac, (lp, ch - 1, jc_l)))
                    # ---- gate nonlinearities + state update ----
                    acts_if = aif_pool.tile([128, 128], F32, tag="aif")
                    nc.scalar.activation(acts_if[:], gps[:, 0:128], AFT.Sigmoid)
                    tg = small_pool.tile([128, 64], F32, tag="tg")
                    nc.scalar.activation(tg[:], gps[:, 128:192], AFT.Tanh)
                    so = small_pool.tile([128, 64], F32, tag="so")
                    nc.scalar.activation(so[:], gps[:, 192:256], AFT.Sigmoid)
                    c_new = c_pool.tile([128, 64], F32, tag="c")
                    nc.vector.tensor_mul(c_new[:], acts_if[:, 64:128], c_prev[:])
                    t1 = small_pool.tile([128, 64], F32, tag="t1")
                    nc.vector.tensor_mul(t1[:], acts_if[:, 0:64], tg[:])
                    nc.vector.tensor_add(c_new[:], c_new[:], t1[:])
                    tcl = small_pool.tile([128, 64], F32, tag="tc")
                    nc.scalar.activation(tcl[:], c_new[:], AFT.Tanh)
                    nc.vector.tensor_mul(ring[:, tl * 64:(tl + 1) * 64],
                                         so[:], tcl[:])
                    c_prev = c_new
                    # deferred evacuations: after the chain in the ACT FIFO
                    for fn, args in pend_evacs:
                        fn(*args)
                    pend_evacs = []
                prev_ring = ring
                if ch + 1 < chunks:
                    xg_cur = xg_next
            # epilogue: linear for the last chunk
            for jc in range(4):
                lp = linear_mms(prev_ring, jc)
                linear_evac(lp, chunks - 1, jc)

    nc.compile()
    return nc


@functools.lru_cache(maxsize=4)
def _get_program(chunks=None):
    return _build_program(chunks)


def _pack_core_inputs(x, h0, c0, Wih, Whh, bias, W21, b21_or_zero):
    """Host-side layout prep for one core. x:[BL,T,D], h0/c0:[BL,H]."""
    npw = ml_dtypes.bfloat16
    xT = np.ascontiguousarray(
        x.transpose(2, 1, 0).reshape(4, 128, T, BL)).astype(npw)
    wih = np.ascontiguousarray(Wih.T.reshape(4, 128, G)).astype(npw)
    whh = np.ascontiguousarray(Whh.T.reshape(4, 128, G)).astype(npw)
    w21 = np.ascontiguousarray(W21.T.reshape(4, 128, H)).astype(npw)
    biasg = np.ascontiguousarray(bias.reshape(16, 128).T).astype(np.float32)
    b21v = np.ascontiguousarray(b21_or_zero.reshape(4, 128).T).astype(np.float32)
    h0p = np.ascontiguousarray(
        h0.T.reshape(4, 128, BL).transpose(1, 0, 2).reshape(128, 64)).astype(npw)
    c0p = np.ascontiguousarray(
        c0.T.reshape(4, 128, BL).transpose(1, 0, 2).reshape(128, 64)).astype(np.float32)
    ident = np.eye(128, dtype=npw)
    return {"xT": xT, "wih": wih, "whh": whh, "w21": w21, "biasg": biasg,
            "b21": b21v, "h0p": h0p, "c0p": c0p, "ident": ident}


def _make_in_maps(inputs):
    f32 = np.float32
    x_f = np.asarray(inputs["x_f"], f32)
    x_b = np.asarray(inputs["x_b"], f32)
    h0_f, c0_f = np.asarray(inputs["h0_f"], f32), np.asarray(inputs["c0_f"], f32)
    h0_b, c0_b = np.asarray(inputs["h0_b"], f32), np.asarray(inputs["c0_b"], f32)
    Wih_f, Whh_f = np.asarray(inputs["Wih_f"], f32), np.asarray(inputs["Whh_f"], f32)
    Wih_b, Whh_b = np.asarray(inputs["Wih_b"], f32), np.asarray(inputs["Whh_b"], f32)
    bias_f = np.asarray(inputs["bih_f"], f32) + np.asarray(inputs["bhh_f"], f32)
    bias_b = np.asarray(inputs["bih_b"], f32) + np.asarray(inputs["bhh_b"], f32)
    W1, b1 = np.asarray(inputs["W1"], f32), np.asarray(inputs["b1"], f32)
    W2, b2 = np.asarray(inputs["W2"], f32), np.asarray(inputs["b2"], f32)

    W21 = (W2 @ W1).astype(f32)
    b21 = (W2 @ b1 + b2).astype(f32)
    zeros = np.zeros_like(b21)

    in_maps = []
    for r in range(N_CORES):
        d, s = r % 2, r // 2
        sl = slice(s * BL, (s + 1) * BL)
        if d == 0:
            in_maps.append(_pack_core_inputs(
                x_f[sl], h0_f[sl], c0_f[sl], Wih_f, Whh_f, bias_f, W21, b21))
        else:
            in_maps.append(_pack_core_inputs(
                x_b[sl], h0_b[sl], c0_b[sl], Wih_b, Whh_b, bias_b, W21, zeros))
    return in_maps


def _assemble(results):
    out = np.empty((B, T, H), np.float32)
    for s in range(N_CORES // 2):
        sT = results[2 * s]["predT"] + results[2 * s + 1]["predT"]
        out[s * BL:(s + 1) * BL] = sT.reshape(H, T, BL).transpose(2, 1, 0)
    return out.reshape(B * T, H)


def kernel(x_f, x_b, h0_f, c0_f, h0_b, c0_b,
           Wih_f, Whh_f, bih_f, bhh_f,
           Wih_b, Whh_b, bih_b, bhh_b,
           W1, b1, W2, b2):
    in_maps = _make_in_maps(dict(
        x_f=x_f, x_b=x_b, h0_f=h0_f, c0_f=c0_f, h0_b=h0_b, c0_b=c0_b,
        Wih_f=Wih_f, Whh_f=Whh_f, bih_f=bih_f, bhh_f=bhh_f,
        Wih_b=Wih_b, Whh_b=Whh_b, bih_b=bih_b, bhh_b=bhh_b,
        W1=W1, b1=b1, W2=W2, b2=b2))
    nc = _get_program()
    res = run_bass_kernel_spmd(nc, in_maps, core_ids=list(range(N_CORES)))
    return _assemble(res.results)


# revision 4
# speedup vs baseline: 1.0134x; 1.0134x over previous
"""Bidirectional-LSTM Trainium2 kernel (nn_BLSTM).

Problem: B=64,T=512,D=H=512. Two independent LSTMs (forward input x_f,
backward input x_b), outputs summed, then two H x H linears collapsed
into one (W21 = W2 @ W1, b21 = W2 @ b1 + b2).

Sharding (8 cores, SPMD): core r: direction r % 2, batch shard r // 2;
each core runs one direction for BL=16 batches; host sums partial outs.

Step design:
  - xg (input projection + bias, precomputed per 32-step chunk) is
    accumulated into the gate PSUM tile by an identity-weight matmul
    issued before the recurrence matmuls (start=True clears the bank),
    so there is no DVE "psum + xg" add and the activations read PSUM.
  - gate column layout [i | f | g | o]; matmul order i,f then g then o,
    so sigmoid(i,f) starts while g/o matmuls still stream, and the o
    sigmoid sits off the critical path.
  - true Tanh table for g and c (Sigmoid/Tanh/Identity share one
    activation table set), cutting the DVE chain to 4 ops:
    c1 = sig_f*c, t1 = sig_i*tanh_g, c = c1+t1, h = sig_o*tanh(c).
  - proj/linear PSUM evacuations are emitted after the chain so they
    never sit ahead of the step's sigmoid in the ACT FIFO.
"""

import functools
import numpy as np
import ml_dtypes

import concourse.bass as bass
import concourse.tile as tile
from concourse import bacc, mybir
from concourse.bass_utils import run_bass_kernel_spmd

# ---------------- problem constants ----------------
B, T, D, H = 64, 512, 512, 512
G = 4 * H                 # 2048 gate dim
N_CORES = 8
BL = B // (N_CORES // 2)  # 16 local batch per core
TC = 32                   # timesteps per chunk
NCH = T // TC             # chunks

F32 = mybir.dt.float32
BF16 = mybir.dt.bfloat16
AFT = mybir.ActivationFunctionType


def _build_program(chunks=None):
    if chunks is None:
        chunks = NCH
    wdt = BF16
    nc = bacc.Bacc("TRN2", target_bir_lowering=False, debug=False,
                   num_devices=N_CORES)

    xT_d = nc.dram_tensor("xT", [4, 128, T, BL], wdt, kind="ExternalInput").ap()
    wih_d = nc.dram_tensor("wih", [4, 128, G], wdt, kind="ExternalInput").ap()
    whh_d = nc.dram_tensor("whh", [4, 128, G], wdt, kind="ExternalInput").ap()
    w21_d = nc.dram_tensor("w21", [4, 128, H], wdt, kind="ExternalInput").ap()
    biasg_d = nc.dram_tensor("biasg", [128, 16], F32, kind="ExternalInput").ap()
    b21_d = nc.dram_tensor("b21", [128, 4], F32, kind="ExternalInput").ap()
    h0_d = nc.dram_tensor("h0p", [128, 64], wdt, kind="ExternalInput").ap()
    c0_d = nc.dram_tensor("c0p", [128, 64], F32, kind="ExternalInput").ap()
    id_d = nc.dram_tensor("ident", [128, 128], wdt, kind="ExternalInput").ap()
    pred_d = nc.dram_tensor("predT", [H, T * BL], F32, kind="ExternalOutput").ap()

    with tile.TileContext(nc) as tc:
        with (
            tc.tile_pool(name="const", bufs=1) as cpool,
            tc.tile_pool(name="xch", bufs=2) as xch_pool,
            tc.tile_pool(name="xg", bufs=2) as xg_pool,
            tc.tile_pool(name="ring", bufs=2) as ring_pool,
            tc.tile_pool(name="aif", bufs=3) as aif_pool,
            tc.tile_pool(name="small", bufs=4) as small_pool,
            tc.tile_pool(name="cstate", bufs=2) as c_pool,
            tc.tile_pool(name="evac", bufs=2) as evac_pool,
            tc.tile_pool(name="gps", bufs=2, space="PSUM") as gps_pool,
            tc.tile_pool(name="pps", bufs=2, space="PSUM") as pps_pool,
            tc.tile_pool(name="lps", bufs=2, space="PSUM") as lps_pool,
        ):
            # ---- preload constants ----
            whh_sb = cpool.tile([128, 4 * G], wdt, tag="whh")
            wih_sb = cpool.tile([128, 4 * G], wdt, tag="wih")
            w21_sb = cpool.tile([128, 4 * H], wdt, tag="w21")
            ident_sb = cpool.tile([128, 128], wdt, tag="ident")
            biasg_sb = cpool.tile([128, 16], F32, tag="biasg")
            b21_sb = cpool.tile([128, 4], F32, tag="b21")
            h0_sb = cpool.tile([128, 64], wdt, tag="h0")
            c0_sb = cpool.tile([128, 64], F32, tag="c0")
            for kc in range(4):
                nc.gpsimd.dma_start(whh_sb[:, kc * G:(kc + 1) * G], whh_d[kc])
                nc.gpsimd.dma_start(wih_sb[:, kc * G:(kc + 1) * G], wih_d[kc])
                nc.gpsimd.dma_start(w21_sb[:, kc * H:(kc + 1) * H], w21_d[kc])
            nc.gpsimd.dma_start(ident_sb[:], id_d[:])
            nc.gpsimd.dma_start(biasg_sb[:], biasg_d[:])
            nc.gpsimd.dma_start(b21_sb[:], b21_d[:])
            nc.gpsimd.dma_start(h0_sb[:], h0_d[:])
            nc.gpsimd.dma_start(c0_sb[:], c0_d[:])

            # ---- projection helpers ----
            def proj_dma(ch):
                xch = xch_pool.tile([128, 4 * TC * BL], wdt, tag="xch")
                for dc in range(4):
                    nc.gpsimd.dma_start(
                        xch[:, dc * TC * BL:(dc + 1) * TC * BL],
                        xT_d[dc, :, ch * TC:(ch + 1) * TC, :])
                return xch

            def proj_mms(xch, jc):
                pp = pps_pool.tile([128, TC * BL], F32, tag="pp")
                for dc in range(4):
                    nc.tensor.matmul(
                        pp[:],
                        wih_sb[:, dc * G + jc * 128: dc * G + (jc + 1) * 128],
                        xch[:, dc * TC * BL:(dc + 1) * TC * BL],
                        start=(dc == 0), stop=(dc == 3))
                return pp

            def proj_evac(pp, xg, jc):
                off = (jc // 4) * 64 + (jc % 4) * 16
                dst = xg[:].rearrange("p (t c) -> p t c", c=256)[:, :, off:off + 16]
                # DVE, not ACT: keeps the 690ns evacuation out of the ACT
                # FIFO where it can delay the binding-path sigmoids/tanhs.
                nc.vector.tensor_scalar_add(dst, pp[:], biasg_sb[:, jc:jc + 1])

            def linear_mms(ring_src, jc):
                lp = lps_pool.tile([128, TC * BL], F32, tag="lp")
                r3 = ring_src[:].rearrange("p (t c) -> p t c", c=64)
                for kc in range(4):
                    nc.tensor.matmul(
                        lp[:],
                        w21_sb[:, kc * H + jc * 128: kc * H + (jc + 1) * 128],
                        r3[:, :, kc * 16:(kc + 1) * 16],
                        start=(kc == 0), stop=(kc == 3))
                return lp

            def linear_evac(lp, ch_src, jc):
                ev = evac_pool.tile([128, TC * BL], F32, tag="ev")
                nc.vector.tensor_scalar_add(ev[:], lp[:], b21_sb[:, jc:jc + 1])
                nc.gpsimd.dma_start(
                    pred_d[jc * 128:(jc + 1) * 128,
                           ch_src * TC * BL:(ch_src + 1) * TC * BL], ev[:])

            # ---- prologue: project chunk 0 ----
            xch = proj_dma(0)
            xg_cur = xg_pool.tile([128, TC * 256], wdt, tag="xg")
            for jc in range(16):
                proj_evac(proj_mms(xch, jc), xg_cur, jc)

            c_prev = c0_sb
            prev_ring = None
            xg_next = None
            pend_evacs = []   # deferred ACT evacuations (emit after chain)
            for ch in range(chunks):
                ring = ring_pool.tile([128, TC * 64], wdt, tag="ring")
                for tl in range(TC):
                    if tl > 0:
                        hsrc, hoff = ring, (tl - 1) * 64
                    elif ch > 0:
                        hsrc, hoff = prev_ring, (TC - 1) * 64
                    else:
                        hsrc, hoff = h0_sb, 0
                    # ---- gate psum: xg preload + recurrence matmuls ----
                    # col layout: [i(0:64) | f(64:128) | g(128:192) | o(192:256)]
                    gps = gps_pool.tile([128, 256], F32, tag="gps")
                    nc.tensor.matmul(gps[:], ident_sb[:],
                                     xg_cur[:, tl * 256:(tl + 1) * 256],
                                     start=True, stop=False,
                                     skip_group_check=True)
                    # emission order i, f, g, o: sigmoid(i,f) starts after
                    # 32 matmuls, and the o-block matmuls + sigmoid(o) sit
                    # entirely off the critical path (h needs them last).
                    for g_idx in (0, 1, 2, 3):   # i, f, g, o
                        for hc in range(4):
                            jc = g_idx * 4 + hc
                            off = g_idx * 64 + hc * 16
                            for kc in range(4):
                                nc.tensor.matmul(
                                    gps[:, off:off + 16],
                                    whh_sb[:, kc * G + jc * 128: kc * G + (jc + 1) * 128],
                                    hsrc[:, hoff + kc * 16: hoff + (kc + 1) * 16],
                                    start=False,
                                    stop=(hc == 3 and kc == 3),
                                    skip_group_check=True)
                    # ---- filler work (PE; runs during this step's chain) ----
                    if ch + 1 < chunks:
                        if tl == 0:
                            xch = proj_dma(ch + 1)
                            xg_next = xg_pool.tile([128, TC * 256], wdt, tag="xg")
                        # one full projection group per step for tl < 16 so
                        # xg_next is complete well before the chunk boundary
                        # (the next chunk's first xg-load reads all 16 groups).
                        if tl < 16:
                            jc_f = tl
                            pp_cur = pps_pool.tile([128, TC * BL], F32, tag="pp")
                            for dc in range(4):
                                nc.tensor.matmul(
                                    pp_cur[:],
                                    wih_sb[:, dc * G + jc_f * 128: dc * G + (jc_f + 1) * 128],
                                    xch[:, dc * TC * BL:(dc + 1) * TC * BL],
                                    start=(dc == 0), stop=(dc == 3))
                            pend_evacs.append(
                                (proj_evac, (pp_cur, xg_next, jc_f)))
                    if ch >= 1 and tl in (3, 11, 19, 27):
                        jc_l = (tl - 3) // 8
                        lp = linear_mms(prev_ring, jc_l)
                        pend_evacs.append((linear_evac, (lp, ch - 1, jc_l)))
                    # ---- gate nonlinearities + state update ----
                    acts_if = aif_pool.tile([128, 128], F32, tag="aif")
                    nc.scalar.activation(acts_if[:], gps[:, 0:128], AFT.Sigmoid)
                    tg = small_pool.tile([128, 64], F32, tag="tg")
                    nc.scalar.activation(tg[:], gps[:, 128:192], AFT.Tanh)
                    so = small_pool.tile([128, 64], F32, tag="so")
                    nc.scalar.activation(so[:], gps[:, 192:256], AFT.Sigmoid)
                    c_new = c_pool.tile([128, 64], F32, tag="c")
                    nc.vector.tensor_mul(c_new[:], acts_if[:, 64:128], c_prev[:])
                    t1 = small_pool.tile([128, 64], F32, tag="t1")
                    nc.vector.tensor_mul(t1[:], acts_if[:, 0:64], tg[:])
                    nc.vector.tensor_add(c_new[:], c_new[:], t1[:])
                    tcl = small_pool.tile([128, 64], F32, tag="tc")
                    nc.scalar.activation(tcl[:], c_new[:], AFT.Tanh)
                    nc.vector.tensor_mul(ring[:, tl * 64:(tl + 1) * 64],
                                         so[:], tcl[:])
                    c_prev = c_new
                    # deferred evacuations: after the chain in the ACT FIFO
                    for fn, args in pend_evacs:
                        fn(*args)
                    pend_evacs = []
                prev_ring = ring
                if ch + 1 < chunks:
                    xg_cur = xg_next
            # epilogue: linear for the last chunk
            for jc in range(4):
                lp = linear_mms(prev_ring, jc)
                linear_evac(lp, chunks - 1, jc)

    nc.compile()
    return nc


@functools.lru_cache(maxsize=4)
def _get_program(chunks=None):
    return _build_program(chunks)


def _pack_core_inputs(x, h0, c0, Wih, Whh, bias, W21, b21_or_zero):
    """Host-side layout prep for one core. x:[BL,T,D], h0/c0:[BL,H]."""
    npw = ml_dtypes.bfloat16
    xT = np.ascontiguousarray(
        x.transpose(2, 1, 0).reshape(4, 128, T, BL)).astype(npw)
    wih = np.ascontiguousarray(Wih.T.reshape(4, 128, G)).astype(npw)
    whh = np.ascontiguousarray(Whh.T.reshape(4, 128, G)).astype(npw)
    w21 = np.ascontiguousarray(W21.T.reshape(4, 128, H)).astype(npw)
    biasg = np.ascontiguousarray(bias.reshape(16, 128).T).astype(np.float32)
    b21v = np.ascontiguousarray(b21_or_zero.reshape(4, 128).T).astype(np.float32)
    h0p = np.ascontiguousarray(
        h0.T.reshape(4, 128, BL).transpose(1, 0, 2).reshape(128, 64)).astype(npw)
    c0p = np.ascontiguousarray(
        c0.T.reshape(4, 128, BL).transpose(1, 0, 2).reshape(128, 64)).astype(np.float32)
    ident = np.eye(128, dtype=npw)
    return {"xT": xT, "wih": wih, "whh": whh, "w21": w21, "biasg": biasg,
            "b21": b21v, "h0p": h0p, "c0p": c0p, "ident": ident}


def _make_in_maps(inputs):
    f32 = np.float32
    x_f = np.asarray(inputs["x_f"], f32)
    x_b = np.asarray(inputs["x_b"], f32)
    h0_f, c0_f = np.asarray(inputs["h0_f"], f32), np.asarray(inputs["c0_f"], f32)
    h0_b, c0_b = np.asarray(inputs["h0_b"], f32), np.asarray(inputs["c0_b"], f32)
    Wih_f, Whh_f = np.asarray(inputs["Wih_f"], f32), np.asarray(inputs["Whh_f"], f32)
    Wih_b, Whh_b = np.asarray(inputs["Wih_b"], f32), np.asarray(inputs["Whh_b"], f32)
    bias_f = np.asarray(inputs["bih_f"], f32) + np.asarray(inputs["bhh_f"], f32)
    bias_b = np.asarray(inputs["bih_b"], f32) + np.asarray(inputs["bhh_b"], f32)
    W1, b1 = np.asarray(inputs["W1"], f32), np.asarray(inputs["b1"], f32)
    W2, b2 = np.asarray(inputs["W2"], f32), np.asarray(inputs["b2"], f32)

    W21 = (W2 @ W1).astype(f32)
    b21 = (W2 @ b1 + b2).astype(f32)
    zeros = np.zeros_like(b21)

    in_maps = []
    for r in range(N_CORES):
        d, s = r % 2, r // 2
        sl = slice(s * BL, (s + 1) * BL)
        if d == 0:
            in_maps.append(_pack_core_inputs(
                x_f[sl], h0_f[sl], c0_f[sl], Wih_f, Whh_f, bias_f, W21, b21))
        else:
            in_maps.append(_pack_core_inputs(
                x_b[sl], h0_b[sl], c0_b[sl], Wih_b, Whh_b, bias_b, W21, zeros))
    return in_maps


def _assemble(results):
    out = np.empty((B, T, H), np.float32)
    for s in range(N_CORES // 2):
        sT = results[2 * s]["predT"] + results[2 * s + 1]["predT"]
        out[s * BL:(s + 1) * BL] = sT.reshape(H, T, BL).transpose(2, 1, 0)
    return out.reshape(B * T, H)


def kernel(x_f, x_b, h0_f, c0_f, h0_b, c0_b,
           Wih_f, Whh_f, bih_f, bhh_f,
           Wih_b, Whh_b, bih_b, bhh_b,
           W1, b1, W2, b2):
    in_maps = _make_in_maps(dict(
        x_f=x_f, x_b=x_b, h0_f=h0_f, c0_f=c0_f, h0_b=h0_b, c0_b=c0_b,
        Wih_f=Wih_f, Whh_f=Whh_f, bih_f=bih_f, bhh_f=bhh_f,
        Wih_b=Wih_b, Whh_b=Whh_b, bih_b=bih_b, bhh_b=bhh_b,
        W1=W1, b1=b1, W2=W2, b2=b2))
    nc = _get_program()
    res = run_bass_kernel_spmd(nc, in_maps, core_ids=list(range(N_CORES)))
    return _assemble(res.results)
